# revision 1
# baseline (speedup 1.0000x reference)
"""GraphSAGE GNN Bass kernel for TRN2, 8-core SPMD.

Strategy (dst-partitioned graph parallel):
  - Core c owns dst nodes [c*V, (c+1)*V). Feature tables (node-major bf16,
    [NC*VP, H] rows) live in every core's HBM, rebuilt per layer by AllGather.
  - Aggregation: in-edges of owned dsts grouped into NB=4 src-buckets (so
    dma_gather's int16 idx addresses <=32k table rows), sorted by dst, packed
    into 128-slot windows (no dst straddles a window; <= W segs per window).
    dma_gather fetches h[src] (PIECE_W*128 slots per call) -> F[128 slots, H].
    Per window: PE matmul lhsT=M_w [128,W] (0/1), rhs=F_w -> seg sums
    [W, H] in PSUM; G=4 windows pack one 128-partition PSUM tile; staged to
    per-bucket DRAM segarr_b (slot-ordered plain writes; race-free).
  - Combine: per dst, gather its <=NB seg sums back (2nd dma_gather, idx ->
    seg slot or a zero row), sum on DVE, scale by inv_deg, PE-transpose into
    feature-major aggT.
  - Transform: z^T = Wl^T(aggT) + Wr^T(hT) in PSUM (fp32); BN stats reduced
    locally then AllReduced; affine(+ReLU) applied into hT; hT transposed/
    cast bf16, AllGathered into the next layer's table. Classifier per core.
"""

import numpy as np
import ml_dtypes
import concourse.bass as bass
import concourse.tile as tile
from concourse import bacc, mybir
from concourse.masks import make_identity

F32 = mybir.dt.float32
BF16 = mybir.dt.bfloat16
I16 = mybir.dt.int16

NC = 8          # cores
NB = 4          # src buckets
H = 128
W = 32          # max segs per window == M width
G = 4           # windows per PSUM group (G*W == 128)
PIECE_W = 32    # windows per gather piece
EPS = 1e-5


class Cfg:
    def __init__(self, N, E, d_in=12, cgrp_tiles=7):
        assert N % (NC * NB) == 0
        self.N, self.E, self.d_in = N, E, d_in
        self.V = N // NC
        self.VP = ((self.V + 127) // 128) * 128
        self.TROWS = NC * self.VP
        self.BROWS = self.TROWS // NB
        assert self.BROWS <= 32768
        self.NT = self.VP // 128
        self.CGRP = min(cgrp_tiles, self.NT)       # node-tiles per combine group
        self.tf_tiles = [(i, min(512, self.V - i)) for i in range(0, self.V, 512)]


def _wrap16(flat):
    """[L] -> [128, L/16]: element i at [i%16, i//16], replicated to 8x16
    partitions (the Q7 gather kernel reads idxs per 16-partition group)."""
    assert flat.size % 16 == 0
    return np.tile(np.ascontiguousarray(flat.reshape(-1, 16).T), (8, 1))


def _pack_bucket(src_b, dst_b):
    """Pack one (core,bucket) edge set. dst_b are local ids.
    Returns slot_src [S]( -1 pad), slot_j [S](-1 pad), seg_dst [nw*W](-1), nw."""
    order = np.argsort(dst_b, kind="stable")
    s, d = src_b[order], dst_b[order]
    if d.size == 0:
        return np.full(0, -1, np.int64), np.full(0, -1, np.int64), np.full(0, -1, np.int64), 0
    uniq, counts = np.unique(d, return_counts=True)
    n = uniq.size
    w_of = np.empty(n, np.int64)
    j_of = np.empty(n, np.int64)
    start_of = np.empty(n, np.int64)
    cur_w, fill, segs = 0, 0, 0
    for i in range(n):
        g = int(counts[i])
        assert g <= 128, f"bucket degree {g} > 128 unsupported"
        if segs == W or fill + g > 128:
            cur_w += 1
            fill, segs = 0, 0
        w_of[i], j_of[i], start_of[i] = cur_w, segs, cur_w * 128 + fill
        fill += g
        segs += 1
    nw = cur_w + 1
    S = nw * 128
    slot_src = np.full(S, -1, np.int64)
    slot_j = np.full(S, -1, np.int64)
    csum = np.concatenate([[0], np.cumsum(counts)[:-1]])
    pos = np.repeat(start_of, counts) + (np.arange(d.size) - np.repeat(csum, counts))
    slot_src[pos] = s
    slot_j[pos] = np.repeat(j_of, counts)
    seg_dst = np.full(nw * W, -1, np.int64)
    seg_dst[w_of * W + j_of] = uniq
    return slot_src, slot_j, seg_dst, nw


def preprocess(edge_index, cfg: Cfg):
    src = np.asarray(edge_index[0], np.int64)
    dst = np.asarray(edge_index[1], np.int64)
    N, V, VP = cfg.N, cfg.V, cfg.VP
    deg = np.bincount(dst, minlength=N).astype(np.float32)
    inv_deg = (np.float32(1.0) / np.maximum(deg, np.float32(1.0))).astype(np.float32)

    core_of = dst // V
    buck_of = src // (N // NB)
    packs = [[None] * NB for _ in range(NC)]
    for c in range(NC):
        mc = core_of == c
        sc, dc, bc = src[mc], dst[mc] - c * V, buck_of[mc]
        for b in range(NB):
            mb = bc == b
            packs[c][b] = _pack_bucket(sc[mb], dc[mb])

    nw_max = max(p[3] for row in packs for p in row)
    NWb = max(PIECE_W, ((nw_max + PIECE_W - 1) // PIECE_W) * PIECE_W)
    S = NWb * 128
    NPC = NWb // PIECE_W
    NSb = (NWb // G) * 128       # seg-slot rows per bucket (pads included)
    assert NSb + 1 <= 32768, NSb

    def table_row(u):
        return (u // V) * VP + (u % V)

    pre = dict(NWb=NWb, S=S, NPC=NPC, NSb=NSb, inv_deg=inv_deg,
               gidx=[], cidx=[], mmat=[], invd=[])
    for c in range(NC):
        gidx = np.zeros((NB, 128, S // 16), np.int16)
        cidx = np.zeros((NB, 128, VP // 16), np.int16)
        mm = np.zeros((NB * NPC, 128, PIECE_W * W), ml_dtypes.bfloat16)
        for b in range(NB):
            slot_src, slot_j, seg_dst, nw = packs[c][b]
            ss = np.full(S, -1, np.int64)
            ss[: slot_src.size] = slot_src
            sj = np.full(S, -1, np.int64)
            sj[: slot_j.size] = slot_j
            rows = np.zeros(S, np.int64)
            val = ss >= 0
            rows[val] = table_row(ss[val]) - b * cfg.BROWS
            assert (rows >= 0).all() and (rows < cfg.BROWS).all()
            gidx[b] = _wrap16(rows.astype(np.int16))
            # M one-hot
            sl = np.nonzero(val)[0]
            wg = sl // 128                      # window
            p = sl % 128
            piece = wg // PIECE_W
            w_in = wg % PIECE_W
            mm[b * NPC + piece, p, w_in * W + sj[sl]] = 1.0
            # combine idx: dst -> seg slot
            cvals = np.full(VP, NSb, np.int64)
            sd = np.full(NWb * W, -1, np.int64)
            sd[: seg_dst.size] = seg_dst
            ok = sd >= 0
            wi = np.arange(NWb * W) // W
            ji = np.arange(NWb * W) % W
            slot_of_seg = (wi // G) * 128 + (wi % G) * W + ji
            cvals[sd[ok]] = slot_of_seg[ok]
            cidx[b] = _wrap16(cvals.astype(np.int16))
        pre["gidx"].append(gidx)
        pre["cidx"].append(cidx)
        pre["mmat"].append(mm)
        it = np.ones((128, cfg.NT), np.float32)
        vr = np.arange(VP)
        vv = vr < V
        it[vr[vv] % 128, vr[vv] // 128] = inv_deg[c * V + vr[vv]]
        pre["invd"].append(np.ascontiguousarray(it))
    return pre


def build_inputs(inputs, pre, cfg: Cfg):
    """inputs: dict from reference.setup_inputs() (numpy). Returns in_maps."""
    N, V, VP, d_in = cfg.N, cfg.V, cfg.VP, cfg.d_in
    x = np.asarray(inputs["x"], np.float32)
    # node-major padded bf16 table for layer 0
    tbl0 = np.zeros((cfg.TROWS, H), ml_dtypes.bfloat16)
    for c in range(NC):
        tbl0[c * VP: c * VP + V, :d_in] = x[c * V:(c + 1) * V]
    pad = lambda a, shp: np.zeros(shp, np.float32) if a is None else a

    def padT(w, rows, cols):  # w [r0, c0] -> [rows, cols] zero-padded
        o = np.zeros((rows, cols), np.float32)
        o[: w.shape[0], : w.shape[1]] = w
        return o

    Wl0 = np.asarray(inputs["Wl0"], np.float32)   # [H, d_in]
    Wr0 = np.asarray(inputs["Wr0"], np.float32)
    Wl = np.asarray(inputs["Wl"], np.float32)     # [2, H, H]
    Wr = np.asarray(inputs["Wr"], np.float32)
    wlT = np.stack([padT(Wl0.T, H, H), Wl[0].T, Wl[1].T]).astype(np.float32)
    wrT = np.stack([padT(Wr0.T, H, H), Wr[0].T, Wr[1].T]).astype(np.float32)
    gam = np.ascontiguousarray(np.asarray(inputs["gamma"], np.float32).T)  # [H,3]
    bet = np.ascontiguousarray(np.asarray(inputs["beta"], np.float32).T)
    wc1T = np.ascontiguousarray(np.asarray(inputs["Wc1"], np.float32).T)   # [H,64]
    bc1 = np.asarray(inputs["bc1"], np.float32).reshape(-1, 1)             # [64,1]
    wc2T = np.ascontiguousarray(np.asarray(inputs["Wc2"], np.float32).T)   # [64,1]
    bc2 = np.asarray(inputs["bc2"], np.float32).reshape(1, 1)

    in_maps = []
    for c in range(NC):
        xT = np.zeros((H, VP), np.float32)
        xT[:d_in, :V] = x[c * V:(c + 1) * V].T
        in_maps.append(dict(
            tbl0=tbl0, xT=xT,
            gidx=pre["gidx"][c], cidx=pre["cidx"][c], mmat=pre["mmat"][c],
            invd=pre["invd"][c],
            wlT=wlT, wrT=wrT, gam=gam, bet=bet,
            wc1T=wc1T, bc1=bc1, wc2T=wc2T, bc2=bc2,
        ))
    return in_maps


def build_program(cfg: Cfg, pre, stop=None, layers=3):
    N, V, VP, NT = cfg.N, cfg.V, cfg.VP, cfg.NT
    NWb, S, NPC, NSb = pre["NWb"], pre["S"], pre["NPC"], pre["NSb"]
    NSR = NSb + 16               # segarr rows (zero row at NSb)
    GRP_PER_PIECE = PIECE_W // G

    nc = bacc.Bacc("TRN2", target_bir_lowering=False, debug=False, num_devices=NC)

    # ---- external I/O ----
    ext = {}
    def ein(name, shape, dt):
        ext[name] = nc.dram_tensor(name, shape, dt, kind="ExternalInput")
        return ext[name]

    tbl0 = ein("tbl0", [cfg.TROWS, H], BF16)
    xT_e = ein("xT", [H, VP], F32)
    gidx_e = ein("gidx", [NB, 128, S // 16], I16)
    cidx_e = ein("cidx", [NB, 128, VP // 16], I16)
    mmat_e = ein("mmat", [NB * NPC, 128, PIECE_W * W], BF16)
    invd_e = ein("invd", [128, NT], F32)
    wlT_e = ein("wlT", [3, H, H], F32)
    wrT_e = ein("wrT", [3, H, H], F32)
    gam_e = ein("gam", [H, 3], F32)
    bet_e = ein("bet", [H, 3], F32)
    wc1T_e = ein("wc1T", [H, 64], F32)
    bc1_e = ein("bc1", [64, 1], F32)
    wc2T_e = ein("wc2T", [64, 1], F32)
    bc2_e = ein("bc2", [1, 1], F32)
    logits_e = nc.dram_tensor("logits", [1, VP], F32, kind="ExternalOutput")
    dbg_e = nc.dram_tensor("dbg", [128, VP], F32, kind="ExternalOutput") if stop else None

    # ---- internal DRAM ----
    segarr = [nc.dram_tensor(f"segarr{b}", [NSR, H], F32) for b in range(NB)]
    tbls = [tbl0,
            nc.dram_tensor("tbl1", [cfg.TROWS, H], BF16, addr_space="Shared"),
            nc.dram_tensor("tbl2", [cfg.TROWS, H], BF16, addr_space="Shared")]
    agin = [None,
            nc.dram_tensor("agin1", [VP, H], BF16),
            nc.dram_tensor("agin2", [VP, H], BF16)]
    zt_d = nc.dram_tensor("zt_d", [H, VP], F32)
    arin = [nc.dram_tensor(f"arin{l}", [H, 2], F32) for l in range(3)]
    arout = [nc.dram_tensor(f"arout{l}", [H, 2], F32, addr_space="Shared")
             for l in range(3)]
    rg = [list(range(NC))]

    with tile.TileContext(nc) as tc:
        import contextlib
        cm = contextlib.ExitStack()
        with cm:
            singles = cm.enter_context(tc.tile_pool(name="singles", bufs=1))
            persist = cm.enter_context(tc.tile_pool(name="persist", bufs=1))
            shared1 = cm.enter_context(tc.tile_pool(name="shared1", bufs=1))
            fpool = cm.enter_context(tc.tile_pool(name="fpool", bufs=3))
            mpool = cm.enter_context(tc.tile_pool(name="mpool", bufs=3))
            stagp = cm.enter_context(tc.tile_pool(name="stagp", bufs=3))
            cpool = cm.enter_context(tc.tile_pool(name="cpool", bufs=6))
            small = cm.enter_context(tc.tile_pool(name="small", bufs=4))
            scr = cm.enter_context(tc.tile_pool(name="scr", bufs=2))
            ps_seg = cm.enter_context(tc.tile_pool(name="ps_seg", bufs=4, space="PSUM"))
            ps_big = cm.enter_context(tc.tile_pool(name="ps_big", bufs=2, space="PSUM"))
            ps_tr = cm.enter_context(tc.tile_pool(name="ps_tr", bufs=2, space="PSUM"))

            # ---- constants ----
            wlT = singles.tile([H, 3, H], F32, tag="wlT")
            wrT = singles.tile([H, 3, H], F32, tag="wrT")
            nc.sync.dma_start(out=wlT[:], in_=wlT_e[:].rearrange("l k m -> k l m"))
            nc.sync.dma_start(out=wrT[:], in_=wrT_e[:].rearrange("l k m -> k l m"))
            gam = singles.tile([H, 3], F32, tag="gam")
            bet = singles.tile([H, 3], F32, tag="bet")
            nc.sync.dma_start(out=gam[:], in_=gam_e[:])
            nc.sync.dma_start(out=bet[:], in_=bet_e[:])
            wc1T = singles.tile([H, 64], F32, tag="wc1T")
            nc.sync.dma_start(out=wc1T[:], in_=wc1T_e[:])
            bc1 = singles.tile([64, 1], F32, tag="bc1")
            nc.sync.dma_start(out=bc1[:], in_=bc1_e[:])
            wc2T = singles.tile([64, 1], F32, tag="wc2T")
            nc.sync.dma_start(out=wc2T[:], in_=wc2T_e[:])
            bc2 = singles.tile([1, 1], F32, tag="bc2")
            nc.sync.dma_start(out=bc2[:], in_=bc2_e[:])
            invd = singles.tile([128, NT], F32, tag="invd")
            nc.sync.dma_start(out=invd[:], in_=invd_e[:])

            cidx = singles.tile([128, NB, VP // 16], I16, tag="cidx")
            nc.sync.dma_start(out=cidx[:], in_=cidx_e[:].rearrange("b p s -> p b s"))
            ident = singles.tile([128, 128], F32, tag="ident")
            make_identity(nc, ident[:])
            ones = singles.tile([128, 512], F32, tag="ones")
            nc.vector.memset(ones[:], 1.0)
            epsT = singles.tile([128, 1], F32, tag="epsT")
            nc.vector.memset(epsT[:], EPS)
            zrow = singles.tile([16, H], F32, tag="zrow")
            nc.vector.memset(zrow[:], 0.0)
            for b in range(NB):
                nc.sync.dma_start(out=segarr[b][NSb:NSb + 16, :], in_=zrow[:])

            # ---- persistent feature buffers ----
            hT = persist.tile([H, VP], F32, tag="hT")
            nc.vector.memset(hT[:], 0.0)
            nc.sync.dma_start(out=hT[:], in_=xT_e[:])

            for layer in range(layers):
                tbl = tbls[layer]
                # ===== Phase A: gather + segment-sum -> segarr =====
                for b in range(NB):
                    tbl_b = tbl[b * cfg.BROWS:(b + 1) * cfg.BROWS, :]
                    for pc in range(NPC):
                        f_t = fpool.tile([128, PIECE_W, H], BF16, tag="f")
                        m_t = mpool.tile([128, PIECE_W * W], BF16, tag="m")
                        nc.sync.dma_start(out=m_t[:], in_=mmat_e[b * NPC + pc])
                        g0 = (pc * PIECE_W * 128) // 16
                        gp_t = mpool.tile([128, PIECE_W * 128 // 16], I16, tag="gp")
                        nc.sync.dma_start(out=gp_t[:],
                                          in_=gidx_e[b, :, g0: g0 + PIECE_W * 128 // 16])
                        nc.gpsimd.dma_gather(
                            out_ap=f_t[:],
                            in_ap=tbl_b,
                            idxs_ap=gp_t[:],
                            num_idxs=PIECE_W * 128,
                            num_idxs_reg=PIECE_W * 128,
                            elem_size=H,
                            single_packet=False,
                        )
                        if stop == "G":
                            continue
                        stag = stagp.tile([128, GRP_PER_PIECE, H], F32, tag="st")
                        for g in range(GRP_PER_PIECE):
                            pseg = ps_seg.tile([128, H], F32, tag="segp")
                            for k in range(G):
                                w = g * G + k
                                nc.tensor.matmul(
                                    pseg[k * W:(k + 1) * W, :],
                                    m_t[:, w * W:(w + 1) * W],
                                    f_t[:, w, :],
                                    start=True, stop=True,
                                    tile_position=(0, k * W),
                                )
                            nc.vector.tensor_copy(out=stag[:, g, :], in_=pseg[:])
                        r0 = pc * GRP_PER_PIECE * 128
                        nc.sync.dma_start(
                            out=segarr[b][r0: r0 + GRP_PER_PIECE * 128, :]
                            .rearrange("(g p) f -> p g f", p=128),
                            in_=stag[:],
                        )

                if stop in ("A", "G") and layer == layers - 1:
                    if stop == "A":
                        nc.sync.dma_start(out=dbg_e[:, :128], in_=segarr[0][:128, :])
                    lz = small.tile([1, 512], F32, tag="lsb")
                    nc.vector.memset(lz[:], 0.0)
                    nc.sync.dma_start(out=logits_e[:, :512], in_=lz[:])
                    break
                # ===== Phase B: combine + inv_deg + transpose -> aggT =====
                aggT = shared1.tile([H, VP], F32, tag="aggT_stage")
                t = 0
                while t < NT:
                    gt = min(cfg.CGRP, NT - t)
                    ct = [cpool.tile([128, cfg.CGRP, H], F32, tag="ct", name=f"ct{b}") for b in range(NB)]
                    for b in range(NB):
                        c0 = (t * 128) // 16
                        nc.gpsimd.dma_gather(
                            out_ap=ct[b][:, :gt, :],
                            in_ap=segarr[b][:, :],
                            idxs_ap=cidx[:, b, c0: c0 + gt * 128 // 16],
                            num_idxs=gt * 128,
                            num_idxs_reg=gt * 128,
                            elem_size=H,
                            single_packet=False,
                        )
                    nc.vector.tensor_add(ct[0][:, :gt, :], ct[0][:, :gt, :], ct[1][:, :gt, :])
                    nc.vector.tensor_add(ct[2][:, :gt, :], ct[2][:, :gt, :], ct[3][:, :gt, :])
                    nc.vector.tensor_add(ct[0][:, :gt, :], ct[0][:, :gt, :], ct[2][:, :gt, :])
                    for i in range(gt):
                        sc = scr.tile([128, 128], F32, tag="sc")
                        nc.vector.tensor_scalar_mul(sc[:], ct[0][:, i, :],
                                                    invd[:, t + i: t + i + 1])
                        ptr = ps_tr.tile([128, 128], F32, tag="trp")
                        nc.tensor.transpose(out=ptr[:], in_=sc[:], identity=ident[:])
                        nc.vector.tensor_copy(out=aggT[:, (t + i) * 128:(t + i + 1) * 128],
                                              in_=ptr[:])
                    t += gt

                if stop == "B" and layer == layers - 1:
                    nc.sync.dma_start(out=dbg_e[:], in_=aggT[:])
                    break
                # ===== Transform + BN stats =====
                n_tf = len(cfg.tf_tiles)
                if stop != "T0":
                    sums = small.tile([128, n_tf], F32, tag="sums")
                    sumsq = small.tile([128, n_tf], F32, tag="sumsq")
                for ti, (c0, nt) in enumerate(cfg.tf_tiles):
                    pz = ps_big.tile([128, 512], F32, tag="tp")
                    nc.tensor.matmul(pz[:, :nt], wlT[:, layer, :],
                                     aggT[:, c0:c0 + nt], start=True, stop=False)
                    nc.tensor.matmul(pz[:, :nt], wrT[:, layer, :],
                                     hT[:, c0:c0 + nt], start=False, stop=True)
                    zt = scr.tile([128, 512], F32, tag="zt")
                    nc.vector.tensor_copy(out=zt[:, :nt], in_=pz[:, :nt])
                    nc.sync.dma_start(out=zt_d[:, c0:c0 + nt], in_=zt[:, :nt])
                    if stop == "T0":
                        continue
                    nc.vector.reduce_sum(out=sums[:, ti:ti + 1], in_=zt[:, :nt],
                                         axis=mybir.AxisListType.X)
                    sq = scr.tile([128, 512], F32, tag="sq")
                    nc.vector.tensor_mul(sq[:, :nt], zt[:, :nt], zt[:, :nt])
                    nc.vector.reduce_sum(out=sumsq[:, ti:ti + 1], in_=sq[:, :nt],
                                         axis=mybir.AxisListType.X)

                stats2 = small.tile([128, 2], F32, tag="stats2")
                nc.vector.reduce_sum(out=stats2[:, 0:1], in_=sums[:],
                                     axis=mybir.AxisListType.X)
                nc.vector.reduce_sum(out=stats2[:, 1:2], in_=sumsq[:],
                                     axis=mybir.AxisListType.X)
                if stop in ("T", "T0") and layer == layers - 1:
                    break
                nc.sync.dma_start(out=arin[layer][:], in_=stats2[:])
                nc.gpsimd.collective_compute(
                    "AllReduce", mybir.AluOpType.add, replica_groups=rg,
                    ins=[arin[layer][:]], outs=[arout[layer][:]])
                gstat = small.tile([128, 2], F32, tag="gstat")
                nc.sync.dma_start(out=gstat[:], in_=arout[layer][:])
                mean = small.tile([128, 1], F32, tag="mean")
                va = small.tile([128, 1], F32, tag="va")
                aa = small.tile([128, 1], F32, tag="aa")
                cc = small.tile([128, 1], F32, tag="cc")
                nc.vector.tensor_scalar_mul(mean[:], gstat[:, 0:1], 1.0 / N)
                nc.vector.tensor_scalar_mul(va[:], gstat[:, 1:2], 1.0 / N)
                nc.vector.tensor_mul(cc[:], mean[:], mean[:])
                nc.vector.tensor_sub(va[:], va[:], cc[:])
                nc.scalar.activation(out=va[:], in_=va[:],
                                     func=mybir.ActivationFunctionType.Sqrt,
                                     bias=epsT[:], scale=1.0)
                nc.vector.reciprocal(va[:], va[:])
                nc.vector.tensor_mul(aa[:], gam[:, layer:layer + 1], va[:])
                nc.vector.tensor_mul(cc[:], mean[:], aa[:])
                nc.vector.tensor_sub(cc[:], bet[:, layer:layer + 1], cc[:])

                # ===== apply affine (+relu) =====
                for (c0, nt) in cfg.tf_tiles:
                    zt = scr.tile([128, 512], F32, tag="zt")
                    nc.sync.dma_start(out=zt[:, :nt], in_=zt_d[:, c0:c0 + nt])
                    if layer < 2:
                        nc.scalar.activation(out=hT[:, c0:c0 + nt], in_=zt[:, :nt],
                                             func=mybir.ActivationFunctionType.Relu,
                                             bias=cc[:], scale=aa[:])
                    else:
                        nc.vector.tensor_scalar(out=hT[:, c0:c0 + nt],
                                                in0=zt[:, :nt],
                                                scalar1=aa[:], scalar2=cc[:],
                                                op0=mybir.AluOpType.mult,
                                                op1=mybir.AluOpType.add)

                # ===== stage + AllGather next table =====
                if layer < 2:
                    stage = shared1.tile([128, NT, H], BF16, tag="aggT_stage")
                    for t2 in range(NT):
                        ptr = ps_tr.tile([128, 128], F32, tag="trp")
                        nc.tensor.transpose(out=ptr[:], in_=hT[:, t2 * 128:(t2 + 1) * 128],
                                            identity=ident[:])
                        nc.vector.tensor_copy(out=stage[:, t2, :], in_=ptr[:])
                    nc.sync.dma_start(
                        out=agin[layer + 1][:].rearrange("(t p) f -> p t f", p=128),
                        in_=stage[:])
                    nc.gpsimd.collective_compute(
                        "AllGather", mybir.AluOpType.bypass, replica_groups=rg,
                        ins=[agin[layer + 1][:]], outs=[tbls[layer + 1][:]])

            if stop == "APPLY":
                nc.sync.dma_start(out=dbg_e[:], in_=hT[:])
            # ===== classifier =====
            for (c0, nt) in cfg.tf_tiles:
                pc1 = ps_big.tile([128, 512], F32, tag="tp")
                nc.tensor.matmul(pc1[:64, :nt], wc1T[:], hT[:, c0:c0 + nt],
                                 start=True, stop=True)
                h3 = scr.tile([128, 512], F32, tag="sq")
                nc.scalar.activation(out=h3[:64, :nt], in_=pc1[:64, :nt],
                                     func=mybir.ActivationFunctionType.Relu,
                                     bias=bc1[:], scale=1.0)
                pc2 = ps_big.tile([128, 512], F32, tag="tp")
                nc.tensor.matmul(pc2[:1, :nt], wc2T[:], h3[:64, :nt],
                                 start=True, stop=True)
                lsb = small.tile([1, 512], F32, tag="lsb")
                nc.vector.tensor_scalar_add(lsb[:, :nt], pc2[:1, :nt], bc2[:])
                nc.sync.dma_start(out=logits_e[:, c0:c0 + nt], in_=lsb[:, :nt])

    nc.compile()
    return nc


def run_full(inputs, cfg=None, n_cores=NC):
    from concourse.bass_utils import run_bass_kernel_spmd
    if cfg is None:
        cfg = Cfg(N=100000, E=3200000)
    pre = preprocess(np.asarray(inputs["edge_index"]), cfg)
    in_maps = build_inputs(inputs, pre, cfg)
    nc = build_program(cfg, pre)
    res = run_bass_kernel_spmd(nc, in_maps, list(range(n_cores)))
    logits = np.concatenate([res.results[c]["logits"][0, :cfg.V] for c in range(NC)])
    return logits


# ======================= harness entry point =======================
LAST_EXEC_NS = None


def _run_with_retry(nc, in_maps, cores, tries=3):
    from concourse.bass_utils import run_bass_kernel_spmd
    last = None
    for _ in range(tries):
        try:
            return run_bass_kernel_spmd(nc, in_maps, cores)
        except Exception as e:  # transient axon terminal failures
            last = e
    raise last


def kernel(**inputs):
    """Full-input entry: shards across 8 NeuronCores internally."""
    import time
    cfg = Cfg(N=100000, E=3200000)
    edge_index = np.asarray(inputs["edge_index"])
    pre = preprocess(edge_index, cfg)
    in_maps = build_inputs(inputs, pre, cfg)
    nc = build_program(cfg, pre)
    res = _run_with_retry(nc, in_maps, list(range(NC)))
    logits = np.concatenate(
        [np.asarray(res.results[c]["logits"])[0, :cfg.V] for c in range(NC)]
    ).astype(np.float32)
    return logits


def benchmark(inputs, reps=5):
    """Device-resident repeated-run timing. Returns (est_device_ns, logits)."""
    import time
    import jax
    from jax.sharding import Mesh, PartitionSpec, NamedSharding
    from jax.experimental.shard_map import shard_map
    from concourse import bass2jax

    cfg = Cfg(N=100000, E=3200000)
    pre = preprocess(np.asarray(inputs["edge_index"]), cfg)
    in_maps = build_inputs(inputs, pre, cfg)
    nc = build_program(cfg, pre)
    bass2jax.install_neuronx_cc_hook()
    n_cores = NC
    in_names, out_names, out_avals, zero_outs = [], [], [], []
    for alloc in nc.m.functions[0].allocations:
        if not isinstance(alloc, mybir.MemoryLocationSet):
            continue
        name = alloc.memorylocations[0].name
        if alloc.kind == "ExternalInput":
            if nc.partition_id_tensor is not None and name == nc.partition_id_tensor.name:
                continue
            in_names.append(name)
        elif alloc.kind == "ExternalOutput":
            shape = tuple(alloc.tensor_shape)
            dtype = mybir.dt.np(alloc.dtype)
            out_names.append(name)
            out_avals.append(jax.core.ShapedArray(shape, dtype))
            zero_outs.append(np.zeros(shape, dtype))
    n_params = len(in_names)
    all_in_names = in_names + out_names
    if nc.partition_id_tensor is not None:
        all_in_names.append(nc.partition_id_tensor.name)
    donate = tuple(range(n_params, n_params + len(out_names)))

    def _body(*args):
        ops = list(args)
        if nc.partition_id_tensor is not None:
            ops.append(bass2jax.partition_id_tensor())
        return tuple(bass2jax._bass_exec_p.bind(
            *ops, out_avals=tuple(out_avals), in_names=tuple(all_in_names),
            out_names=tuple(out_names), lowering_input_output_aliases=(),
            sim_require_finite=True, sim_require_nnan=True, nc=nc))

    mesh = Mesh(np.asarray(jax.devices()[:n_cores]), ("core",))
    sharded = jax.jit(shard_map(_body, mesh=mesh,
                                in_specs=(PartitionSpec("core"),) * (n_params + len(out_names)),
                                out_specs=(PartitionSpec("core"),) * len(out_names),
                                check_rep=False),
                      donate_argnums=donate, keep_unused=True)
    sh = NamedSharding(mesh, PartitionSpec("core"))
    dev_in = [jax.device_put(np.concatenate(
        [np.asarray(in_maps[c][nm]) for c in range(n_cores)], axis=0), sh)
        for nm in in_names]
    for d in dev_in:
        d.block_until_ready()
    walls = []
    out = None
    for _ in range(reps + 1):
        zeros = [jax.device_put(np.zeros((n_cores * z.shape[0], *z.shape[1:]), z.dtype), sh)
                 for z in zero_outs]
        for z in zeros:
            z.block_until_ready()
        t0 = time.time()
        out = sharded(*dev_in, *zeros)
        for o in out:
            o.block_until_ready()
        walls.append(time.time() - t0)
    best = min(walls[1:])
    # measured axon per-call dispatch floor on this path is ~60-76 ms;
    # report best-wall minus a conservative 60 ms floor (no NTFF in container)
    print(f"per-call walls (s): {[round(w, 4) for w in walls]}")
    est_ns = max(best - 0.060, 0.001) * 1e9
    la = np.asarray(out[out_names.index("logits")]).reshape(n_cores, 1, cfg.VP)
    logits = np.concatenate([la[c, 0, :cfg.V] for c in range(n_cores)]).astype(np.float32)
    return est_ns, logits



# revision 22
# speedup vs baseline: 1.0670x; 1.0670x over previous
"""GraphSAGE GNN Bass kernel for TRN2, 8-core SPMD.

Strategy (dst-partitioned graph parallel):
  - Core c owns dst nodes [c*V, (c+1)*V). Feature tables (node-major bf16,
    [NC*VP, H] rows) live in every core's HBM, rebuilt per layer by AllGather.
  - Aggregation: in-edges of owned dsts grouped into NB=4 src-buckets (so
    dma_gather's int16 idx addresses <=32k table rows), sorted by dst, packed
    into 128-slot windows (no dst straddles a window; <= W segs per window).
    dma_gather fetches h[src] (PIECE_W*128 slots per call) -> F[128 slots, H].
    Per window: PE matmul lhsT=M_w [128,W] (0/1), rhs=F_w -> seg sums
    [W, H] in PSUM; G=4 windows pack one 128-partition PSUM tile; staged to
    per-bucket DRAM segarr_b (slot-ordered plain writes; race-free).
  - Combine: per dst, gather its <=NB seg sums back (2nd dma_gather, idx ->
    seg slot or a zero row), sum on DVE, scale by inv_deg, PE-transpose into
    feature-major aggT.
  - Transform: z^T = Wl^T(aggT) + Wr^T(hT) in PSUM (fp32); BN stats reduced
    locally then AllReduced; affine(+ReLU) applied into hT; hT transposed/
    cast bf16, AllGathered into the next layer's table. Classifier per core.
"""

import numpy as np
import ml_dtypes
import concourse.bass as bass
import concourse.tile as tile
from concourse import bacc, mybir
from concourse.masks import make_identity

F32 = mybir.dt.float32
BF16 = mybir.dt.bfloat16
I16 = mybir.dt.int16

NC = 8          # cores
NB = 4          # src buckets
H = 128
W = 32          # max segs per window == M width
G = 4           # windows per PSUM group (G*W == 128)
PIECE_W = 32    # windows per gather piece
EPS = 1e-5


class Cfg:
    def __init__(self, N, E, d_in=12, cgrp_tiles=7):
        assert N % (NC * NB) == 0
        self.N, self.E, self.d_in = N, E, d_in
        self.V = N // NC
        self.VP = ((self.V + 127) // 128) * 128
        self.TROWS = NC * self.VP
        self.BROWS = self.TROWS // NB
        assert self.BROWS <= 32768
        self.NT = self.VP // 128
        self.CGRP = min(cgrp_tiles, self.NT)       # node-tiles per combine group
        self.tf_tiles = [(i, min(512, self.V - i)) for i in range(0, self.V, 512)]


def _wrap16(flat):
    """[L] -> [128, L/16]: element i at [i%16, i//16], replicated to 8x16
    partitions (the Q7 gather kernel reads idxs per 16-partition group)."""
    assert flat.size % 16 == 0
    return np.tile(np.ascontiguousarray(flat.reshape(-1, 16).T), (8, 1))


def _pack_bucket(src_b, dst_b):
    """Pack one (core,bucket) edge set. dst_b are local ids.
    Returns slot_src [S]( -1 pad), slot_j [S](-1 pad), seg_dst [nw*W](-1), nw."""
    order = np.argsort(dst_b, kind="stable")
    s, d = src_b[order], dst_b[order]
    if d.size == 0:
        return np.full(0, -1, np.int64), np.full(0, -1, np.int64), np.full(0, -1, np.int64), 0
    uniq, counts = np.unique(d, return_counts=True)
    n = uniq.size
    w_of = np.empty(n, np.int64)
    j_of = np.empty(n, np.int64)
    start_of = np.empty(n, np.int64)
    cur_w, fill, segs = 0, 0, 0
    for i in range(n):
        g = int(counts[i])
        assert g <= 128, f"bucket degree {g} > 128 unsupported"
        if segs == W or fill + g > 128:
            cur_w += 1
            fill, segs = 0, 0
        w_of[i], j_of[i], start_of[i] = cur_w, segs, cur_w * 128 + fill
        fill += g
        segs += 1
    nw = cur_w + 1
    S = nw * 128
    slot_src = np.full(S, -1, np.int64)
    slot_j = np.full(S, -1, np.int64)
    csum = np.concatenate([[0], np.cumsum(counts)[:-1]])
    pos = np.repeat(start_of, counts) + (np.arange(d.size) - np.repeat(csum, counts))
    slot_src[pos] = s
    slot_j[pos] = np.repeat(j_of, counts)
    seg_dst = np.full(nw * W, -1, np.int64)
    seg_dst[w_of * W + j_of] = uniq
    return slot_src, slot_j, seg_dst, nw


def preprocess(edge_index, cfg: Cfg):
    src = np.asarray(edge_index[0], np.int64)
    dst = np.asarray(edge_index[1], np.int64)
    N, V, VP = cfg.N, cfg.V, cfg.VP
    deg = np.bincount(dst, minlength=N).astype(np.float32)
    inv_deg = (np.float32(1.0) / np.maximum(deg, np.float32(1.0))).astype(np.float32)

    core_of = dst // V
    buck_of = src // (N // NB)
    packs = [[None] * NB for _ in range(NC)]
    for c in range(NC):
        mc = core_of == c
        sc, dc, bc = src[mc], dst[mc] - c * V, buck_of[mc]
        for b in range(NB):
            mb = bc == b
            packs[c][b] = _pack_bucket(sc[mb], dc[mb])

    nw_max = max(p[3] for row in packs for p in row)
    NWb = max(PIECE_W, ((nw_max + PIECE_W - 1) // PIECE_W) * PIECE_W)
    S = NWb * 128
    NPC = NWb // PIECE_W
    NSb = (NWb // G) * 128       # seg-slot rows per bucket (pads included)
    assert NSb + 1 <= 32768, NSb

    def table_row(u):
        return (u // V) * VP + (u % V)

    pre = dict(NWb=NWb, S=S, NPC=NPC, NSb=NSb, inv_deg=inv_deg,
               gidx=[], cidx=[], mmat=[], invd=[])
    for c in range(NC):
        gidx = np.zeros((NB, 128, S // 16), np.int16)
        cidx = np.zeros((NB, 128, VP // 16), np.int16)
        mm = np.zeros((NB * NPC, 128, PIECE_W * W), ml_dtypes.bfloat16)
        for b in range(NB):
            slot_src, slot_j, seg_dst, nw = packs[c][b]
            ss = np.full(S, -1, np.int64)
            ss[: slot_src.size] = slot_src
            sj = np.full(S, -1, np.int64)
            sj[: slot_j.size] = slot_j
            rows = np.zeros(S, np.int64)
            val = ss >= 0
            rows[val] = table_row(ss[val]) - b * cfg.BROWS
            assert (rows >= 0).all() and (rows < cfg.BROWS).all()
            gidx[b] = _wrap16(rows.astype(np.int16))
            # M one-hot
            sl = np.nonzero(val)[0]
            wg = sl // 128                      # window
            p = sl % 128
            piece = wg // PIECE_W
            w_in = wg % PIECE_W
            mm[b * NPC + piece, p, w_in * W + sj[sl]] = 1.0
            # combine idx: dst -> seg slot
            cvals = np.full(VP, NSb, np.int64)
            sd = np.full(NWb * W, -1, np.int64)
            sd[: seg_dst.size] = seg_dst
            ok = sd >= 0
            wi = np.arange(NWb * W) // W
            ji = np.arange(NWb * W) % W
            slot_of_seg = (wi // G) * 128 + (wi % G) * W + ji
            cvals[sd[ok]] = slot_of_seg[ok]
            cidx[b] = _wrap16(cvals.astype(np.int16))
        pre["gidx"].append(gidx)
        pre["cidx"].append(cidx)
        pre["mmat"].append(mm)
        it = np.ones((128, cfg.NT), np.float32)
        vr = np.arange(VP)
        vv = vr < V
        it[vr[vv] % 128, vr[vv] // 128] = inv_deg[c * V + vr[vv]]
        pre["invd"].append(np.ascontiguousarray(it))
    return pre


def build_inputs(inputs, pre, cfg: Cfg):
    """inputs: dict from reference.setup_inputs() (numpy). Returns in_maps."""
    N, V, VP, d_in = cfg.N, cfg.V, cfg.VP, cfg.d_in
    x = np.asarray(inputs["x"], np.float32)
    # node-major padded bf16 table for layer 0
    tbl0 = np.zeros((cfg.TROWS, H), ml_dtypes.bfloat16)
    for c in range(NC):
        tbl0[c * VP: c * VP + V, :d_in] = x[c * V:(c + 1) * V]
    pad = lambda a, shp: np.zeros(shp, np.float32) if a is None else a

    def padT(w, rows, cols):  # w [r0, c0] -> [rows, cols] zero-padded
        o = np.zeros((rows, cols), np.float32)
        o[: w.shape[0], : w.shape[1]] = w
        return o

    Wl0 = np.asarray(inputs["Wl0"], np.float32)   # [H, d_in]
    Wr0 = np.asarray(inputs["Wr0"], np.float32)
    Wl = np.asarray(inputs["Wl"], np.float32)     # [2, H, H]
    Wr = np.asarray(inputs["Wr"], np.float32)
    wlT = np.stack([padT(Wl0.T, H, H), Wl[0].T, Wl[1].T]).astype(np.float32)
    wrT = np.stack([padT(Wr0.T, H, H), Wr[0].T, Wr[1].T]).astype(np.float32)
    gam = np.ascontiguousarray(np.asarray(inputs["gamma"], np.float32).T)  # [H,3]
    bet = np.ascontiguousarray(np.asarray(inputs["beta"], np.float32).T)
    wc1T = np.ascontiguousarray(np.asarray(inputs["Wc1"], np.float32).T)   # [H,64]
    bc1 = np.asarray(inputs["bc1"], np.float32).reshape(-1, 1)             # [64,1]
    wc2T = np.ascontiguousarray(np.asarray(inputs["Wc2"], np.float32).T)   # [64,1]
    bc2 = np.asarray(inputs["bc2"], np.float32).reshape(1, 1)

    in_maps = []
    for c in range(NC):
        xT = np.zeros((H, VP), np.float32)
        xT[:d_in, :V] = x[c * V:(c + 1) * V].T
        in_maps.append(dict(
            tbl0=tbl0, xT=xT,
            gidx=pre["gidx"][c], cidx=pre["cidx"][c], mmat=pre["mmat"][c],
            invd=pre["invd"][c],
            wlT=wlT, wrT=wrT, gam=gam, bet=bet,
            wc1T=wc1T, bc1=bc1, wc2T=wc2T, bc2=bc2,
        ))
    return in_maps


def build_program(cfg: Cfg, pre, stop=None, layers=3):
    N, V, VP, NT = cfg.N, cfg.V, cfg.VP, cfg.NT
    NWb, S, NPC, NSb = pre["NWb"], pre["S"], pre["NPC"], pre["NSb"]
    NSR = NSb + 16               # segarr rows (zero row at NSb)
    GRP_PER_PIECE = PIECE_W // G

    nc = bacc.Bacc("TRN2", target_bir_lowering=False, debug=False, num_devices=NC)

    # ---- external I/O ----
    ext = {}
    def ein(name, shape, dt):
        ext[name] = nc.dram_tensor(name, shape, dt, kind="ExternalInput")
        return ext[name]

    tbl0 = ein("tbl0", [cfg.TROWS, H], BF16)
    xT_e = ein("xT", [H, VP], F32)
    gidx_e = ein("gidx", [NB, 128, S // 16], I16)
    cidx_e = ein("cidx", [NB, 128, VP // 16], I16)
    mmat_e = ein("mmat", [NB * NPC, 128, PIECE_W * W], BF16)
    invd_e = ein("invd", [128, NT], F32)
    wlT_e = ein("wlT", [3, H, H], F32)
    wrT_e = ein("wrT", [3, H, H], F32)
    gam_e = ein("gam", [H, 3], F32)
    bet_e = ein("bet", [H, 3], F32)
    wc1T_e = ein("wc1T", [H, 64], F32)
    bc1_e = ein("bc1", [64, 1], F32)
    wc2T_e = ein("wc2T", [64, 1], F32)
    bc2_e = ein("bc2", [1, 1], F32)
    logits_e = nc.dram_tensor("logits", [1, VP], F32, kind="ExternalOutput")
    dbg_e = (nc.dram_tensor("dbg", [128, VP], F32, kind="ExternalOutput")
             if stop in ("A", "B", "APPLY") else None)

    # ---- internal DRAM ----
    segarr = [nc.dram_tensor(f"segarr{b}", [NSR, H], F32) for b in range(NB)]
    tbls = [tbl0,
            nc.dram_tensor("tbl1", [cfg.TROWS, H], BF16, addr_space="Shared"),
            nc.dram_tensor("tbl2", [cfg.TROWS, H], BF16, addr_space="Shared")]
    agin = [None,
            nc.dram_tensor("agin1", [VP, H], BF16),
            nc.dram_tensor("agin2", [VP, H], BF16)]
    zt_d = nc.dram_tensor("zt_d", [H, VP], F32)
    arin = [nc.dram_tensor(f"arin{l}", [H, 2], F32) for l in range(3)]
    arout = [nc.dram_tensor(f"arout{l}", [H, 2], F32, addr_space="Shared")
             for l in range(3)]
    rg = [list(range(NC))]

    with tile.TileContext(nc) as tc:
        import contextlib
        cm = contextlib.ExitStack()
        with cm:
            singles = cm.enter_context(tc.tile_pool(name="singles", bufs=1))
            persist = cm.enter_context(tc.tile_pool(name="persist", bufs=1))
            shared1 = cm.enter_context(tc.tile_pool(name="shared1", bufs=1))
            fpool = cm.enter_context(tc.tile_pool(name="fpool", bufs=3))
            mpool = cm.enter_context(tc.tile_pool(name="mpool", bufs=3))
            stagp = cm.enter_context(tc.tile_pool(name="stagp", bufs=3))
            cpool = cm.enter_context(tc.tile_pool(name="cpool", bufs=6))
            small = cm.enter_context(tc.tile_pool(name="small", bufs=4))
            scr = cm.enter_context(tc.tile_pool(name="scr", bufs=2))
            ps_seg = cm.enter_context(tc.tile_pool(name="ps_seg", bufs=4, space="PSUM"))
            ps_big = cm.enter_context(tc.tile_pool(name="ps_big", bufs=2, space="PSUM"))
            ps_tr = cm.enter_context(tc.tile_pool(name="ps_tr", bufs=2, space="PSUM"))

            # ---- constants ----
            wlT = singles.tile([H, 3, H], F32, tag="wlT")
            wrT = singles.tile([H, 3, H], F32, tag="wrT")
            nc.sync.dma_start(out=wlT[:], in_=wlT_e[:].rearrange("l k m -> k l m"))
            nc.sync.dma_start(out=wrT[:], in_=wrT_e[:].rearrange("l k m -> k l m"))
            gam = singles.tile([H, 3], F32, tag="gam")
            bet = singles.tile([H, 3], F32, tag="bet")
            nc.sync.dma_start(out=gam[:], in_=gam_e[:])
            nc.sync.dma_start(out=bet[:], in_=bet_e[:])
            wc1T = singles.tile([H, 64], F32, tag="wc1T")
            nc.sync.dma_start(out=wc1T[:], in_=wc1T_e[:])
            bc1 = singles.tile([64, 1], F32, tag="bc1")
            nc.sync.dma_start(out=bc1[:], in_=bc1_e[:])
            wc2T = singles.tile([64, 1], F32, tag="wc2T")
            nc.sync.dma_start(out=wc2T[:], in_=wc2T_e[:])
            bc2 = singles.tile([1, 1], F32, tag="bc2")
            nc.sync.dma_start(out=bc2[:], in_=bc2_e[:])
            invd = singles.tile([128, NT], F32, tag="invd")
            nc.sync.dma_start(out=invd[:], in_=invd_e[:])

            cidx = singles.tile([128, NB, VP // 16], I16, tag="cidx")
            nc.sync.dma_start(out=cidx[:], in_=cidx_e[:].rearrange("b p s -> p b s"))
            ident = singles.tile([128, 128], F32, tag="ident")
            make_identity(nc, ident[:])
            ones = singles.tile([128, 512], F32, tag="ones")
            nc.vector.memset(ones[:], 1.0)
            epsT = singles.tile([128, 1], F32, tag="epsT")
            nc.vector.memset(epsT[:], EPS)
            zrow = singles.tile([16, H], F32, tag="zrow")
            nc.vector.memset(zrow[:], 0.0)
            for b in range(NB):
                nc.sync.dma_start(out=segarr[b][NSb:NSb + 16, :], in_=zrow[:])

            # ---- persistent feature buffers ----
            hT = persist.tile([H, VP], F32, tag="hT")
            nc.vector.memset(hT[:], 0.0)
            nc.sync.dma_start(out=hT[:], in_=xT_e[:])

            for layer in range(layers):
                tbl = tbls[layer]
                # ===== Phase A: gather + segment-sum -> segarr =====
                for b in range(NB):
                    tbl_b = tbl[b * cfg.BROWS:(b + 1) * cfg.BROWS, :]
                    for pc in range(NPC):
                        f_t = fpool.tile([128, PIECE_W, H], BF16, tag="f")
                        if stop != "Gm":
                            m_t = mpool.tile([128, PIECE_W * W], BF16, tag="m")
                            nc.sync.dma_start(out=m_t[:], in_=mmat_e[b * NPC + pc])
                        g0 = (pc * PIECE_W * 128) // 16
                        gp_t = mpool.tile([128, PIECE_W * 128 // 16], I16, tag="gp")
                        nc.sync.dma_start(out=gp_t[:],
                                          in_=gidx_e[b, :, g0: g0 + PIECE_W * 128 // 16])
                        nc.gpsimd.dma_gather(
                            out_ap=f_t[:],
                            in_ap=tbl_b,
                            idxs_ap=gp_t[:],
                            num_idxs=PIECE_W * 128,
                            num_idxs_reg=PIECE_W * 128,
                            elem_size=H,
                            single_packet=False,
                        )
                        if stop in ("G", "Gm"):
                            continue
                        stag = stagp.tile([128, GRP_PER_PIECE, H], F32, tag="st")
                        for g in range(GRP_PER_PIECE):
                            pseg = ps_seg.tile([128, H], F32, tag="segp")
                            for k in range(G):
                                w = g * G + k
                                nc.tensor.matmul(
                                    pseg[k * W:(k + 1) * W, :],
                                    m_t[:, w * W:(w + 1) * W],
                                    f_t[:, w, :],
                                    start=True, stop=True,
                                    tile_position=(0, k * W),
                                )
                            nc.vector.tensor_copy(out=stag[:, g, :], in_=pseg[:])
                        r0 = pc * GRP_PER_PIECE * 128
                        nc.sync.dma_start(
                            out=segarr[b][r0: r0 + GRP_PER_PIECE * 128, :]
                            .rearrange("(g p) f -> p g f", p=128),
                            in_=stag[:],
                        )

                if stop in ("A", "At", "G", "Gm") and layer == layers - 1:
                    if stop == "A":
                        nc.sync.dma_start(out=dbg_e[:, :128], in_=segarr[0][:128, :])
                    lz = small.tile([1, 512], F32, tag="lsb")
                    nc.vector.memset(lz[:], 0.0)
                    nc.sync.dma_start(out=logits_e[:, :512], in_=lz[:])
                    break
                # ===== Phase B: combine + inv_deg + transpose -> aggT =====
                aggT = shared1.tile([H, VP], F32, tag="aggT_stage")
                t = 0
                while t < NT:
                    gt = min(cfg.CGRP, NT - t)
                    ct = [cpool.tile([128, cfg.CGRP, H], F32, tag="ct", name=f"ct{b}") for b in range(NB)]
                    for b in range(NB):
                        c0 = (t * 128) // 16
                        nc.gpsimd.dma_gather(
                            out_ap=ct[b][:, :gt, :],
                            in_ap=segarr[b][:, :],
                            idxs_ap=cidx[:, b, c0: c0 + gt * 128 // 16],
                            num_idxs=gt * 128,
                            num_idxs_reg=gt * 128,
                            elem_size=H,
                            single_packet=False,
                        )
                    nc.vector.tensor_add(ct[0][:, :gt, :], ct[0][:, :gt, :], ct[1][:, :gt, :])
                    nc.vector.tensor_add(ct[2][:, :gt, :], ct[2][:, :gt, :], ct[3][:, :gt, :])
                    nc.vector.tensor_add(ct[0][:, :gt, :], ct[0][:, :gt, :], ct[2][:, :gt, :])
                    for i in range(gt):
                        sc = scr.tile([128, 128], F32, tag="sc")
                        nc.vector.tensor_scalar_mul(sc[:], ct[0][:, i, :],
                                                    invd[:, t + i: t + i + 1])
                        ptr = ps_tr.tile([128, 128], F32, tag="trp")
                        nc.tensor.transpose(out=ptr[:], in_=sc[:], identity=ident[:])
                        nc.vector.tensor_copy(out=aggT[:, (t + i) * 128:(t + i + 1) * 128],
                                              in_=ptr[:])
                    t += gt

                if stop in ("B", "Bt") and layer == layers - 1:
                    if stop == "B":
                        nc.sync.dma_start(out=dbg_e[:], in_=aggT[:])
                    break
                # ===== Transform + BN stats =====
                n_tf = len(cfg.tf_tiles)
                if stop != "T0":
                    sums = small.tile([128, n_tf], F32, tag="sums")
                    sumsq = small.tile([128, n_tf], F32, tag="sumsq")
                for ti, (c0, nt) in enumerate(cfg.tf_tiles):
                    pz = ps_big.tile([128, 512], F32, tag="tp")
                    nc.tensor.matmul(pz[:, :nt], wlT[:, layer, :],
                                     aggT[:, c0:c0 + nt], start=True, stop=False)
                    nc.tensor.matmul(pz[:, :nt], wrT[:, layer, :],
                                     hT[:, c0:c0 + nt], start=False, stop=True)
                    zt = scr.tile([128, 512], F32, tag="zt")
                    nc.vector.tensor_copy(out=zt[:, :nt], in_=pz[:, :nt])
                    nc.sync.dma_start(out=zt_d[:, c0:c0 + nt], in_=zt[:, :nt])
                    if stop == "T0":
                        continue
                    nc.vector.reduce_sum(out=sums[:, ti:ti + 1], in_=zt[:, :nt],
                                         axis=mybir.AxisListType.X)
                    sq = scr.tile([128, 512], F32, tag="sq")
                    nc.vector.tensor_mul(sq[:, :nt], zt[:, :nt], zt[:, :nt])
                    nc.vector.reduce_sum(out=sumsq[:, ti:ti + 1], in_=sq[:, :nt],
                                         axis=mybir.AxisListType.X)

                stats2 = small.tile([128, 2], F32, tag="stats2")
                nc.vector.reduce_sum(out=stats2[:, 0:1], in_=sums[:],
                                     axis=mybir.AxisListType.X)
                nc.vector.reduce_sum(out=stats2[:, 1:2], in_=sumsq[:],
                                     axis=mybir.AxisListType.X)
                if stop in ("T", "T0") and layer == layers - 1:
                    break
                nc.sync.dma_start(out=arin[layer][:], in_=stats2[:])
                nc.gpsimd.collective_compute(
                    "AllReduce", mybir.AluOpType.add, replica_groups=rg,
                    ins=[arin[layer][:]], outs=[arout[layer][:]])
                gstat = small.tile([128, 2], F32, tag="gstat")
                nc.sync.dma_start(out=gstat[:], in_=arout[layer][:])
                mean = small.tile([128, 1], F32, tag="mean")
                va = small.tile([128, 1], F32, tag="va")
                aa = small.tile([128, 1], F32, tag="aa")
                cc = small.tile([128, 1], F32, tag="cc")
                nc.vector.tensor_scalar_mul(mean[:], gstat[:, 0:1], 1.0 / N)
                nc.vector.tensor_scalar_mul(va[:], gstat[:, 1:2], 1.0 / N)
                nc.vector.tensor_mul(cc[:], mean[:], mean[:])
                nc.vector.tensor_sub(va[:], va[:], cc[:])
                nc.scalar.activation(out=va[:], in_=va[:],
                                     func=mybir.ActivationFunctionType.Sqrt,
                                     bias=epsT[:], scale=1.0)
                nc.vector.reciprocal(va[:], va[:])
                nc.vector.tensor_mul(aa[:], gam[:, layer:layer + 1], va[:])
                nc.vector.tensor_mul(cc[:], mean[:], aa[:])
                nc.vector.tensor_sub(cc[:], bet[:, layer:layer + 1], cc[:])

                # ===== apply affine (+relu) =====
                for (c0, nt) in cfg.tf_tiles:
                    zt = scr.tile([128, 512], F32, tag="zt")
                    nc.sync.dma_start(out=zt[:, :nt], in_=zt_d[:, c0:c0 + nt])
                    if layer < 2:
                        nc.scalar.activation(out=hT[:, c0:c0 + nt], in_=zt[:, :nt],
                                             func=mybir.ActivationFunctionType.Relu,
                                             bias=cc[:], scale=aa[:])
                    else:
                        nc.vector.tensor_scalar(out=hT[:, c0:c0 + nt],
                                                in0=zt[:, :nt],
                                                scalar1=aa[:], scalar2=cc[:],
                                                op0=mybir.AluOpType.mult,
                                                op1=mybir.AluOpType.add)

                # ===== stage + AllGather next table =====
                if layer < 2:
                    stage = shared1.tile([128, NT, H], BF16, tag="aggT_stage")
                    for t2 in range(NT):
                        ptr = ps_tr.tile([128, 128], F32, tag="trp")
                        nc.tensor.transpose(out=ptr[:], in_=hT[:, t2 * 128:(t2 + 1) * 128],
                                            identity=ident[:])
                        nc.vector.tensor_copy(out=stage[:, t2, :], in_=ptr[:])
                    nc.sync.dma_start(
                        out=agin[layer + 1][:].rearrange("(t p) f -> p t f", p=128),
                        in_=stage[:])
                    nc.gpsimd.collective_compute(
                        "AllGather", mybir.AluOpType.bypass, replica_groups=rg,
                        ins=[agin[layer + 1][:]], outs=[tbls[layer + 1][:]])

            if stop == "APPLY":
                nc.sync.dma_start(out=dbg_e[:], in_=hT[:])
            # ===== classifier =====
            for (c0, nt) in cfg.tf_tiles:
                pc1 = ps_big.tile([128, 512], F32, tag="tp")
                nc.tensor.matmul(pc1[:64, :nt], wc1T[:], hT[:, c0:c0 + nt],
                                 start=True, stop=True)
                h3 = scr.tile([128, 512], F32, tag="sq")
                nc.scalar.activation(out=h3[:64, :nt], in_=pc1[:64, :nt],
                                     func=mybir.ActivationFunctionType.Relu,
                                     bias=bc1[:], scale=1.0)
                pc2 = ps_big.tile([128, 512], F32, tag="tp")
                nc.tensor.matmul(pc2[:1, :nt], wc2T[:], h3[:64, :nt],
                                 start=True, stop=True)
                lsb = small.tile([1, 512], F32, tag="lsb")
                nc.vector.tensor_scalar_add(lsb[:, :nt], pc2[:1, :nt], bc2[:])
                nc.sync.dma_start(out=logits_e[:, c0:c0 + nt], in_=lsb[:, :nt])

    nc.compile()
    return nc


def run_full(inputs, cfg=None, n_cores=NC):
    from concourse.bass_utils import run_bass_kernel_spmd
    if cfg is None:
        cfg = Cfg(N=100000, E=3200000)
    pre = preprocess(np.asarray(inputs["edge_index"]), cfg)
    in_maps = build_inputs(inputs, pre, cfg)
    nc = build_program(cfg, pre)
    res = run_bass_kernel_spmd(nc, in_maps, list(range(n_cores)))
    logits = np.concatenate([res.results[c]["logits"][0, :cfg.V] for c in range(NC)])
    return logits


# ======================= harness entry point =======================
LAST_EXEC_NS = None


def _run_with_retry(nc, in_maps, cores, tries=3):
    from concourse.bass_utils import run_bass_kernel_spmd
    last = None
    for _ in range(tries):
        try:
            return run_bass_kernel_spmd(nc, in_maps, cores)
        except Exception as e:  # transient axon terminal failures
            last = e
    raise last


def kernel(**inputs):
    """Full-input entry: shards across 8 NeuronCores internally."""
    import time
    cfg = Cfg(N=100000, E=3200000)
    edge_index = np.asarray(inputs["edge_index"])
    pre = preprocess(edge_index, cfg)
    in_maps = build_inputs(inputs, pre, cfg)
    nc = build_program(cfg, pre)
    res = _run_with_retry(nc, in_maps, list(range(NC)))
    logits = np.concatenate(
        [np.asarray(res.results[c]["logits"])[0, :cfg.V] for c in range(NC)]
    ).astype(np.float32)
    return logits


def benchmark(inputs, reps=5):
    """Device-resident repeated-run timing. Returns (est_device_ns, logits)."""
    import time
    import jax
    from jax.sharding import Mesh, PartitionSpec, NamedSharding
    from jax.experimental.shard_map import shard_map
    from concourse import bass2jax

    cfg = Cfg(N=100000, E=3200000)
    pre = preprocess(np.asarray(inputs["edge_index"]), cfg)
    in_maps = build_inputs(inputs, pre, cfg)
    nc = build_program(cfg, pre)
    bass2jax.install_neuronx_cc_hook()
    n_cores = NC
    in_names, out_names, out_avals, zero_outs = [], [], [], []
    for alloc in nc.m.functions[0].allocations:
        if not isinstance(alloc, mybir.MemoryLocationSet):
            continue
        name = alloc.memorylocations[0].name
        if alloc.kind == "ExternalInput":
            if nc.partition_id_tensor is not None and name == nc.partition_id_tensor.name:
                continue
            in_names.append(name)
        elif alloc.kind == "ExternalOutput":
            shape = tuple(alloc.tensor_shape)
            dtype = mybir.dt.np(alloc.dtype)
            out_names.append(name)
            out_avals.append(jax.core.ShapedArray(shape, dtype))
            zero_outs.append(np.zeros(shape, dtype))
    n_params = len(in_names)
    all_in_names = in_names + out_names
    if nc.partition_id_tensor is not None:
        all_in_names.append(nc.partition_id_tensor.name)
    donate = tuple(range(n_params, n_params + len(out_names)))

    def _body(*args):
        ops = list(args)
        if nc.partition_id_tensor is not None:
            ops.append(bass2jax.partition_id_tensor())
        return tuple(bass2jax._bass_exec_p.bind(
            *ops, out_avals=tuple(out_avals), in_names=tuple(all_in_names),
            out_names=tuple(out_names), lowering_input_output_aliases=(),
            sim_require_finite=True, sim_require_nnan=True, nc=nc))

    mesh = Mesh(np.asarray(jax.devices()[:n_cores]), ("core",))
    sharded = jax.jit(shard_map(_body, mesh=mesh,
                                in_specs=(PartitionSpec("core"),) * (n_params + len(out_names)),
                                out_specs=(PartitionSpec("core"),) * len(out_names),
                                check_rep=False),
                      donate_argnums=donate, keep_unused=True)
    sh = NamedSharding(mesh, PartitionSpec("core"))
    dev_in = [jax.device_put(np.concatenate(
        [np.asarray(in_maps[c][nm]) for c in range(n_cores)], axis=0), sh)
        for nm in in_names]
    for d in dev_in:
        d.block_until_ready()
    walls = []
    out = None
    for _ in range(reps + 1):
        zeros = [jax.device_put(np.zeros((n_cores * z.shape[0], *z.shape[1:]), z.dtype), sh)
                 for z in zero_outs]
        for z in zeros:
            z.block_until_ready()
        t0 = time.time()
        out = sharded(*dev_in, *zeros)
        for o in out:
            o.block_until_ready()
        walls.append(time.time() - t0)
    best = min(walls[1:])
    # measured axon per-call dispatch floor on this path is ~60-76 ms;
    # report best-wall minus a conservative 60 ms floor (no NTFF in container)
    print(f"per-call walls (s): {[round(w, 4) for w in walls]}")
    est_ns = max(best - 0.060, 0.001) * 1e9
    la = np.asarray(out[out_names.index("logits")]).reshape(n_cores, 1, cfg.VP)
    logits = np.concatenate([la[c, 0, :cfg.V] for c in range(n_cores)]).astype(np.float32)
    return est_ns, logits

